# revision 6
# baseline (speedup 1.0000x reference)
"""Trainium2 Bass kernel for CubeFaceNN.

Computes, for x of shape [8, 1, 128, 128, 128] (f32):
    out[b, i, p] = relu(x[b, 0, p] - x[b, 0, p + OFF[i]])   (zero padded)
with OFF = [(0,-1,-1), (-1,0,-1), (1,-1,-1), (-1,1,-1), (-1,-1,0), (-1,-1,1)]
(derived from the reference's adj % 3 - 1 indexing).

Sharding: pure data parallel — batch b -> NeuronCore b (8 cores).

Per-core layout: depth d on the 128 SBUF partitions, (h, w) in the free
dims. x is fully resident in SBUF (64KB/partition); a partition-shifted
copy xp[d] = x[d+1] is loaded straight from HBM (compute engines cannot
address SBUF at a partition offset of 1). Channels with od = -1 are
computed in the substituted frame out[i, d'+1] = relu(xp[d'] - x[d',
h+oh, w+ow]) so one shifted copy serves all five d-shifting channels;
the d-boundary faces are written from small [h, w]-layout plane tiles.

All large transfers use SWDGE (nc.gpsimd): the HWDGE dynamic ring drains
through one SDMA engine (~27 GB/s), while SWDGE swizzles descriptors
across all 16. 128-partition SWDGE transfers take a 4-engine split path,
so those are peeled into 127+1 partitions.
"""

import numpy as np

import concourse.bacc as bacc
import concourse.mybir as mybir
import concourse.tile as tile
from concourse.bass_utils import run_bass_kernel_spmd

D = H = W = 128
N_CORES = 8
HC = 32  # h-chunk size for output stores
F32 = mybir.dt.float32

# (od, oh, ow) per output channel
OFFSETS = [(0, -1, -1), (-1, 0, -1), (1, -1, -1), (-1, 1, -1), (-1, -1, 0), (-1, -1, 1)]

_NC_CACHE = {}


def build_nc(debug=False):
    nc = bacc.Bacc("TRN2", target_bir_lowering=False, debug=debug)
    x = nc.dram_tensor("x", [D, H, W], F32, kind="ExternalInput")
    out = nc.dram_tensor("out", [6, D, H, W], F32, kind="ExternalOutput")

    sub = mybir.AluOpType.subtract
    relu = mybir.ActivationFunctionType.Relu

    with tile.TileContext(nc) as tc:
        with (
            tc.tile_pool(name="xt", bufs=1) as xt_pool,
            tc.tile_pool(name="xp", bufs=1) as xp_pool,
            tc.tile_pool(name="och", bufs=3) as och_pool,
            tc.tile_pool(name="plane", bufs=2) as plane_pool,
        ):
            xt = xt_pool.tile([D, H, W], F32)
            nc.gpsimd.dma_start(out=xt[0 : D - 1], in_=x[0 : D - 1])
            nc.gpsimd.dma_start(out=xt[D - 1 : D], in_=x[D - 1 : D])
            # xp[d] = x[d+1] on partitions 0..126, straight from HBM
            xp = xp_pool.tile([D, H, W], F32)
            nc.gpsimd.dma_start(out=xp[0 : D - 1], in_=x[1:D])

            # d-boundary planes: out[i, 0] = relu(x[0]) for od=-1 channels,
            # out[2, 127] = relu(x[127]). Loaded with h on partitions so the
            # relu runs on 128 partitions. HWDGE (sync) ring: tiny transfers,
            # keeps the SWDGE queue for the big ones.
            p0 = plane_pool.tile([H, W], F32)
            nc.sync.dma_start(out=p0[:], in_=x[0])
            nc.vector.tensor_scalar_max(p0[:], p0[:], 0.0)
            for i, (od, _, _) in enumerate(OFFSETS):
                if od == -1:
                    nc.sync.dma_start(out=out[i, 0], in_=p0[:])
            p1 = plane_pool.tile([H, W], F32)
            nc.sync.dma_start(out=p1[:], in_=x[D - 1])
            nc.vector.tensor_scalar_max(p1[:], p1[:], 0.0)
            nc.sync.dma_start(out=out[2, D - 1], in_=p1[:])

            for c in range(H // HC):
                h0 = c * HC
                for i, (od, oh, ow) in enumerate(OFFSETS):
                    # A = operand aligned with the output partition frame,
                    # S = the d-shifted operand (reads at h+oh, w+ow).
                    if od == -1:
                        A, S = xp, xt  # substituted frame
                    elif od == 1:
                        A, S = xt, xp
                    else:
                        A, S = xt, xt
                    dc = D if od == 0 else D - 1

                    # valid output rows/cols (shifted source in range)
                    hs = max(h0, -oh)
                    he = min(h0 + HC, H - max(0, oh))
                    ws = max(0, -ow)
                    we = W - max(0, ow)

                    och = och_pool.tile([D, HC, W], F32)
                    nc.vector.tensor_tensor(
                        out=och[0:dc, hs - h0 : he - h0, ws:we],
                        in0=A[0:dc, hs:he, ws:we],
                        in1=S[0:dc, hs + oh : he + oh, ws + ow : we + ow],
                        op=sub,
                    )
                    nc.scalar.activation(
                        och[0:dc, hs - h0 : he - h0, ws:we],
                        och[0:dc, hs - h0 : he - h0, ws:we],
                        relu,
                    )
                    # boundary strips: shifted source is zero there -> relu(A)
                    if oh == -1 and h0 == 0:
                        nc.vector.tensor_scalar_max(
                            och[0:dc, 0:1, :], A[0:dc, 0:1, :], 0.0
                        )
                    if oh == 1 and h0 + HC == H:
                        nc.vector.tensor_scalar_max(
                            och[0:dc, HC - 1 : HC, :], A[0:dc, H - 1 : H, :], 0.0
                        )
                    if ow != 0:
                        wb = 0 if ow == -1 else W - 1
                        nc.vector.tensor_scalar_max(
                            och[0:dc, hs - h0 : he - h0, wb : wb + 1],
                            A[0:dc, hs:he, wb : wb + 1],
                            0.0,
                        )

                    if od == -1:
                        dst = out[i, 1:D, h0 : h0 + HC, :]
                    elif od == 1:
                        dst = out[i, 0 : D - 1, h0 : h0 + HC, :]
                    else:
                        # peel the last partition: 127-partition stores
                        # spread over all 16 SDMA engines, 128 do not.
                        dst = out[i, 0 : D - 1, h0 : h0 + HC, :]
                        nc.gpsimd.dma_start(
                            out=out[i, D - 1 : D, h0 : h0 + HC, :],
                            in_=och[D - 1 : D],
                        )
                    nc.gpsimd.dma_start(out=dst, in_=och[0 : D - 1])

    nc.compile()
    return nc


def _get_nc():
    if "nc" not in _NC_CACHE:
        _NC_CACHE["nc"] = build_nc()
    return _NC_CACHE["nc"]


def kernel(x: np.ndarray) -> np.ndarray:
    assert x.shape == (N_CORES, 1, D, H, W), x.shape
    nc = _get_nc()
    in_maps = [{"x": np.ascontiguousarray(x[b, 0], dtype=np.float32)} for b in range(N_CORES)]
    res = run_bass_kernel_spmd(nc, in_maps, core_ids=list(range(N_CORES)))
    return np.stack([r["out"] for r in res.results], axis=0)


# revision 7
# speedup vs baseline: 1.8731x; 1.8731x over previous
"""Trainium2 Bass kernel for CubeFaceNN.

Computes, for x of shape [8, 1, 128, 128, 128] (f32):
    out[b, i, p] = relu(x[b, 0, p] - x[b, 0, p + OFF[i]])   (zero padded)
with OFF = [(0,-1,-1), (-1,0,-1), (1,-1,-1), (-1,1,-1), (-1,-1,0), (-1,-1,1)]
(derived from the reference's adj % 3 - 1 indexing).

Sharding: pure data parallel — batch b -> NeuronCore b (8 cores).

Per-core layout: depth d on the 128 SBUF partitions, (h, w) in the free
dims. x is fully resident in SBUF (64KB/partition); a partition-shifted
copy xp[d] = x[d+1] is loaded straight from HBM in h-chunks (compute
engines cannot address SBUF at a partition offset of 1). Channels with
od = -1 are computed in the substituted frame
    out[i, d'+1] = relu(xp[d'] - x[d', h+oh, w+ow])
so one shifted copy serves all five d-shifting channels; the d-boundary
faces are written from small [h, w]-layout plane tiles.

DMA rules learned from traces on this silicon:
  - The HWDGE dynamic ring drains through a single SDMA engine
    (~27 GB/s) regardless of size -> only tiny plane transfers use
    nc.sync.
  - SWDGE (nc.gpsimd) swizzles descriptors across all 16 engines only
    when the per-partition contiguous run is <= 16 KB; larger runs land
    on one engine. All big transfers use exactly 32 rows x 512 B = 16 KB
    runs.
  - 128-partition SWDGE transfers split across only 4 engines;
    127-partition ones use all 16. Big transfers are peeled to 127+1.
"""

import numpy as np

import concourse.bacc as bacc
import concourse.mybir as mybir
import concourse.tile as tile
from concourse.bass_utils import run_bass_kernel_spmd

D = H = W = 128
N_CORES = 8
HC = 32  # h-chunk size
F32 = mybir.dt.float32

# (od, oh, ow) per output channel
OFFSETS = [(0, -1, -1), (-1, 0, -1), (1, -1, -1), (-1, 1, -1), (-1, -1, 0), (-1, -1, 1)]

_NC_CACHE = {}


def build_nc(debug=False):
    nc = bacc.Bacc("TRN2", target_bir_lowering=False, debug=debug)
    x = nc.dram_tensor("x", [D, H, W], F32, kind="ExternalInput")
    out = nc.dram_tensor("out", [6, D, H, W], F32, kind="ExternalOutput")

    sub = mybir.AluOpType.subtract
    relu = mybir.ActivationFunctionType.Relu
    n_chunks = H // HC

    with tile.TileContext(nc) as tc:
        with (
            tc.tile_pool(name="xt", bufs=1) as xt_pool,
            tc.tile_pool(name="xp", bufs=2) as xp_pool,
            tc.tile_pool(name="och", bufs=5) as och_pool,
            tc.tile_pool(name="plane", bufs=2) as plane_pool,
        ):
            # x fully resident, loaded as 4 chunks x (127+1 partitions)
            xt = xt_pool.tile([D, H, W], F32)
            for c in range(n_chunks):
                hsl = slice(c * HC, (c + 1) * HC)
                nc.gpsimd.dma_start(out=xt[0 : D - 1, hsl, :], in_=x[0 : D - 1, hsl, :])
                nc.gpsimd.dma_start(out=xt[D - 1 : D, hsl, :], in_=x[D - 1 : D, hsl, :])

            def load_xp_chunk(c):
                # xp rows needed for chunk c: [c*HC - 1, c*HC + HC) on
                # partitions 0..126 (xp[d, r] = x[d+1, lo + r]).
                lo = max(0, c * HC - 1)
                hi = c * HC + HC
                t = xp_pool.tile([D, HC + 1, W], F32)
                base = 1 if c > 0 else 0  # local row of absolute row c*HC
                # main 32-row (16 KB) piece + <=1-row tail to stay <=16 KB
                nc.gpsimd.dma_start(
                    out=t[0 : D - 1, 0:HC, :], in_=x[1:D, lo : lo + HC, :]
                )
                if hi - lo > HC:
                    nc.gpsimd.dma_start(
                        out=t[0 : D - 1, HC : HC + 1, :],
                        in_=x[1:D, lo + HC : hi, :],
                    )
                return t, base

            xp_tiles = {0: load_xp_chunk(0)}

            # d-boundary planes: out[i, 0] = relu(x[0]) for od=-1 channels,
            # out[2, 127] = relu(x[127]); h on partitions so relu is wide.
            p0 = plane_pool.tile([H, W], F32)
            nc.sync.dma_start(out=p0[:], in_=x[0])
            nc.vector.tensor_scalar_max(p0[:], p0[:], 0.0)
            for i, (od, _, _) in enumerate(OFFSETS):
                if od == -1:
                    nc.sync.dma_start(out=out[i, 0], in_=p0[:])
            p1 = plane_pool.tile([H, W], F32)
            nc.sync.dma_start(out=p1[:], in_=x[D - 1])
            nc.vector.tensor_scalar_max(p1[:], p1[:], 0.0)
            nc.sync.dma_start(out=out[2, D - 1], in_=p1[:])

            for c in range(n_chunks):
                h0 = c * HC
                xp, xpb = xp_tiles.pop(c)
                if c + 1 < n_chunks:  # prefetch before this chunk's stores
                    xp_tiles[c + 1] = load_xp_chunk(c + 1)

                def xprow(h):  # absolute h row -> local xp row
                    return h - h0 + xpb

                for i, (od, oh, ow) in enumerate(OFFSETS):
                    # A = operand aligned with the output partition frame,
                    # S = the d-shifted operand (reads at h+oh, w+ow).
                    dc = D if od == 0 else D - 1

                    hs = max(h0, -oh)
                    he = min(h0 + HC, H - max(0, oh))
                    ws = max(0, -ow)
                    we = W - max(0, ow)

                    if od == -1:  # substituted frame: A=xp, S=xt
                        in0 = xp[0:dc, xprow(hs) : xprow(he), ws:we]
                        in1 = xt[0:dc, hs + oh : he + oh, ws + ow : we + ow]
                    elif od == 1:  # A=xt, S=xp
                        in0 = xt[0:dc, hs:he, ws:we]
                        in1 = xp[
                            0:dc, xprow(hs + oh) : xprow(he + oh), ws + ow : we + ow
                        ]
                    else:
                        in0 = xt[0:dc, hs:he, ws:we]
                        in1 = xt[0:dc, hs + oh : he + oh, ws + ow : we + ow]

                    och = och_pool.tile([D, HC, W], F32)
                    nc.vector.tensor_tensor(
                        out=och[0:dc, hs - h0 : he - h0, ws:we],
                        in0=in0,
                        in1=in1,
                        op=sub,
                    )
                    # boundary strips (shifted source zero there -> relu(A));
                    # on ACT so the store depends on one engine's tail only.
                    def strip_src(hb_s, hb_e, wb_s, wb_e):
                        if od == -1:
                            return xp[0:dc, xprow(hb_s) : xprow(hb_e), wb_s:wb_e]
                        return xt[0:dc, hb_s:hb_e, wb_s:wb_e]

                    if oh == -1 and h0 == 0:
                        nc.scalar.activation(
                            och[0:dc, 0:1, :], strip_src(0, 1, 0, W), relu
                        )
                    if oh == 1 and h0 + HC == H:
                        nc.scalar.activation(
                            och[0:dc, HC - 1 : HC, :], strip_src(H - 1, H, 0, W), relu
                        )
                    if ow != 0:
                        wb = 0 if ow == -1 else W - 1
                        nc.scalar.activation(
                            och[0:dc, hs - h0 : he - h0, wb : wb + 1],
                            strip_src(hs, he, wb, wb + 1),
                            relu,
                        )
                    nc.scalar.activation(
                        och[0:dc, hs - h0 : he - h0, ws:we],
                        och[0:dc, hs - h0 : he - h0, ws:we],
                        relu,
                    )

                    if od == -1:
                        dst = out[i, 1:D, h0 : h0 + HC, :]
                    elif od == 1:
                        dst = out[i, 0 : D - 1, h0 : h0 + HC, :]
                    else:
                        # peel last partition so the store spreads 16-wide
                        dst = out[i, 0 : D - 1, h0 : h0 + HC, :]
                        nc.gpsimd.dma_start(
                            out=out[i, D - 1 : D, h0 : h0 + HC, :],
                            in_=och[D - 1 : D],
                        )
                    nc.gpsimd.dma_start(out=dst, in_=och[0 : D - 1])

    nc.compile()
    return nc


def _get_nc():
    if "nc" not in _NC_CACHE:
        _NC_CACHE["nc"] = build_nc()
    return _NC_CACHE["nc"]


def kernel(x: np.ndarray) -> np.ndarray:
    assert x.shape == (N_CORES, 1, D, H, W), x.shape
    nc = _get_nc()
    in_maps = [{"x": np.ascontiguousarray(x[b, 0], dtype=np.float32)} for b in range(N_CORES)]
    res = run_bass_kernel_spmd(nc, in_maps, core_ids=list(range(N_CORES)))
    return np.stack([r["out"] for r in res.results], axis=0)


# revision 8
# speedup vs baseline: 2.2081x; 1.1789x over previous
"""Trainium2 Bass kernel for CubeFaceNN.

Computes, for x of shape [8, 1, 128, 128, 128] (f32):
    out[b, i, p] = relu(x[b, 0, p] - x[b, 0, p + OFF[i]])   (zero padded)
with OFF = [(0,-1,-1), (-1,0,-1), (1,-1,-1), (-1,1,-1), (-1,-1,0), (-1,-1,1)]
(derived from the reference's adj % 3 - 1 indexing).

Sharding: pure data parallel — batch b -> NeuronCore b (8 cores).

Per-core layout: depth d on the 128 SBUF partitions, (h, w) in the free
dims. x is fully resident in SBUF (64KB/partition); a partition-shifted
copy xp[d] = x[d+1] is loaded straight from HBM in h-chunks (compute
engines cannot address SBUF at a partition offset of 1). Channels with
od = -1 are computed in the substituted frame
    out[i, d'+1] = relu(xp[d'] - x[d', h+oh, w+ow])
so one shifted copy serves all five d-shifting channels; the d-boundary
faces are written from small [h, w]-layout plane tiles.

DMA rules learned from traces on this silicon:
  - The HWDGE dynamic ring drains through a single SDMA engine
    (~27 GB/s) regardless of size -> only tiny transfers use nc.sync.
  - SWDGE (nc.gpsimd) swizzles a transfer's descriptors across all 16
    engines only when the per-partition contiguous run is <= 16 KB.
  - 128-partition SWDGE transfers split across only 4 engines;
    127-partition ones use all 16 -> big transfers are peeled to 127+1,
    with the 1-partition remainder on the HWDGE ring.
  - Store chunks must be small enough (HC=16 rows -> 8 KB runs) and the
    och pool deep enough (bufs=10 > 6 channels/chunk) that compute of
    chunk c+1 never waits on stores of chunk c.
"""

import numpy as np

import concourse.bacc as bacc
import concourse.mybir as mybir
import concourse.tile as tile
from concourse.bass_utils import run_bass_kernel_spmd

D = H = W = 128
N_CORES = 8
HC = 16  # h-chunk size for compute/stores
XC = 32  # h-chunk size for xp loads
F32 = mybir.dt.float32

# (od, oh, ow) per output channel
OFFSETS = [(0, -1, -1), (-1, 0, -1), (1, -1, -1), (-1, 1, -1), (-1, -1, 0), (-1, -1, 1)]

_NC_CACHE = {}


def build_nc(debug=False):
    nc = bacc.Bacc("TRN2", target_bir_lowering=False, debug=debug)
    x = nc.dram_tensor("x", [D, H, W], F32, kind="ExternalInput")
    out = nc.dram_tensor("out", [6, D, H, W], F32, kind="ExternalOutput")

    sub = mybir.AluOpType.subtract
    relu = mybir.ActivationFunctionType.Relu
    n_chunks = H // HC

    with tile.TileContext(nc) as tc:
        with (
            tc.tile_pool(name="xt", bufs=1) as xt_pool,
            tc.tile_pool(name="xp", bufs=2) as xp_pool,
            tc.tile_pool(name="och", bufs=10) as och_pool,
            tc.tile_pool(name="plane", bufs=2) as plane_pool,
        ):
            # x fully resident, loaded as 4 x (127 partitions, 32 rows)
            # SWDGE pieces + 1-partition remainders on the HWDGE ring.
            xt = xt_pool.tile([D, H, W], F32)
            for c in range(H // XC):
                hsl = slice(c * XC, (c + 1) * XC)
                nc.gpsimd.dma_start(out=xt[0 : D - 1, hsl, :], in_=x[0 : D - 1, hsl, :])
                nc.sync.dma_start(out=xt[D - 1 : D, hsl, :], in_=x[D - 1 : D, hsl, :])

            def load_xp_chunk(cx):
                # xp rows [cx*XC - 1, cx*XC + XC) on partitions 0..126
                # (xp[d, r] = x[d+1, lo + r]); 32-row (16 KB) SWDGE main
                # piece + <=1-row tail on the HWDGE ring.
                lo = max(0, cx * XC - 1)
                hi = cx * XC + XC
                t = xp_pool.tile([D, XC + 1, W], F32)
                base = 1 if cx > 0 else 0  # local row of absolute row cx*XC
                nc.gpsimd.dma_start(
                    out=t[0 : D - 1, 0:XC, :], in_=x[1:D, lo : lo + XC, :]
                )
                if hi - lo > XC:
                    nc.sync.dma_start(
                        out=t[0 : D - 1, XC : XC + 1, :],
                        in_=x[1:D, lo + XC : hi, :],
                    )
                return t, base

            xp_tiles = {0: load_xp_chunk(0)}

            # d-boundary planes: out[i, 0] = relu(x[0]) for od=-1 channels,
            # out[2, 127] = relu(x[127]); h on partitions so relu is wide.
            p0 = plane_pool.tile([H, W], F32)
            nc.sync.dma_start(out=p0[:], in_=x[0])
            nc.vector.tensor_scalar_max(p0[:], p0[:], 0.0)
            for i, (od, _, _) in enumerate(OFFSETS):
                if od == -1:
                    nc.sync.dma_start(out=out[i, 0], in_=p0[:])
            p1 = plane_pool.tile([H, W], F32)
            nc.sync.dma_start(out=p1[:], in_=x[D - 1])
            nc.vector.tensor_scalar_max(p1[:], p1[:], 0.0)
            nc.sync.dma_start(out=out[2, D - 1], in_=p1[:])

            for c in range(n_chunks):
                h0 = c * HC
                cx = h0 // XC  # xp tile covering this compute chunk
                if h0 % XC == 0:
                    xp, xpb = xp_tiles.pop(cx)
                    if cx + 1 < H // XC:  # prefetch before this chunk's stores
                        xp_tiles[cx + 1] = load_xp_chunk(cx + 1)
                x0 = cx * XC  # absolute row of xp local row `xpb`

                def xprow(h):  # absolute h row -> local xp row
                    return h - x0 + xpb

                for i, (od, oh, ow) in enumerate(OFFSETS):
                    # A = operand aligned with the output partition frame,
                    # S = the d-shifted operand (reads at h+oh, w+ow).
                    dc = D if od == 0 else D - 1

                    hs = max(h0, -oh)
                    he = min(h0 + HC, H - max(0, oh))
                    ws = max(0, -ow)
                    we = W - max(0, ow)

                    if od == -1:  # substituted frame: A=xp, S=xt
                        in0 = xp[0:dc, xprow(hs) : xprow(he), ws:we]
                        in1 = xt[0:dc, hs + oh : he + oh, ws + ow : we + ow]
                    elif od == 1:  # A=xt, S=xp
                        in0 = xt[0:dc, hs:he, ws:we]
                        in1 = xp[
                            0:dc, xprow(hs + oh) : xprow(he + oh), ws + ow : we + ow
                        ]
                    else:
                        in0 = xt[0:dc, hs:he, ws:we]
                        in1 = xt[0:dc, hs + oh : he + oh, ws + ow : we + ow]

                    och = och_pool.tile([D, HC, W], F32)
                    nc.vector.tensor_tensor(
                        out=och[0:dc, hs - h0 : he - h0, ws:we],
                        in0=in0,
                        in1=in1,
                        op=sub,
                    )
                    # boundary strips (shifted source zero there -> relu(A));
                    # on ACT so the store depends on one engine's tail only.
                    def strip_src(hb_s, hb_e, wb_s, wb_e):
                        if od == -1:
                            return xp[0:dc, xprow(hb_s) : xprow(hb_e), wb_s:wb_e]
                        return xt[0:dc, hb_s:hb_e, wb_s:wb_e]

                    if oh == -1 and h0 == 0:
                        nc.scalar.activation(
                            och[0:dc, 0:1, :], strip_src(0, 1, 0, W), relu
                        )
                    if oh == 1 and h0 + HC == H:
                        nc.scalar.activation(
                            och[0:dc, HC - 1 : HC, :], strip_src(H - 1, H, 0, W), relu
                        )
                    if ow != 0:
                        wb = 0 if ow == -1 else W - 1
                        nc.scalar.activation(
                            och[0:dc, hs - h0 : he - h0, wb : wb + 1],
                            strip_src(hs, he, wb, wb + 1),
                            relu,
                        )
                    nc.scalar.activation(
                        och[0:dc, hs - h0 : he - h0, ws:we],
                        och[0:dc, hs - h0 : he - h0, ws:we],
                        relu,
                    )

                    if od == -1:
                        dst = out[i, 1:D, h0 : h0 + HC, :]
                    elif od == 1:
                        dst = out[i, 0 : D - 1, h0 : h0 + HC, :]
                    else:
                        # peel last partition so the store spreads 16-wide;
                        # remainder goes on the HWDGE ring.
                        dst = out[i, 0 : D - 1, h0 : h0 + HC, :]
                        nc.sync.dma_start(
                            out=out[i, D - 1 : D, h0 : h0 + HC, :],
                            in_=och[D - 1 : D],
                        )
                    nc.gpsimd.dma_start(out=dst, in_=och[0 : D - 1])

    nc.compile()
    return nc


def _get_nc():
    if "nc" not in _NC_CACHE:
        _NC_CACHE["nc"] = build_nc()
    return _NC_CACHE["nc"]


def kernel(x: np.ndarray) -> np.ndarray:
    assert x.shape == (N_CORES, 1, D, H, W), x.shape
    nc = _get_nc()
    in_maps = [{"x": np.ascontiguousarray(x[b, 0], dtype=np.float32)} for b in range(N_CORES)]
    res = run_bass_kernel_spmd(nc, in_maps, core_ids=list(range(N_CORES)))
    return np.stack([r["out"] for r in res.results], axis=0)


# revision 9
# speedup vs baseline: 4.0960x; 1.8550x over previous
"""Trainium2 Bass kernel for CubeFaceNN.

Computes, for x of shape [8, 1, 128, 128, 128] (f32):
    out[b, i, p] = relu(x[b, 0, p] - x[b, 0, p + OFF[i]])   (zero padded)
with OFF = [(0,-1,-1), (-1,0,-1), (1,-1,-1), (-1,1,-1), (-1,-1,0), (-1,-1,1)]
(derived from the reference's adj % 3 - 1 indexing).

Sharding: pure data parallel — batch b -> NeuronCore b (8 cores).

Per-core layout: depth d on the 128 SBUF partitions, (h, w) in the free
dims. x is fully resident in SBUF (64KB/partition); a partition-shifted
copy xp[d] = x[d+1] is loaded straight from HBM in h-chunks (compute
engines cannot address SBUF at a partition offset of 1). Channels with
od = -1 are computed in the substituted frame
    out[i, d'+1] = relu(xp[d'] - x[d', h+oh, w+ow])
so one shifted copy serves all five d-shifting channels; the d-boundary
faces are written from small [h, w]-layout plane tiles.

DMA rules learned from traces/probes on this silicon:
  - The HWDGE dynamic ring drains through a single SDMA engine
    (~27 GB/s) -> only tiny plane/tail transfers use nc.sync.
  - SWDGE (nc.gpsimd) spreads descriptors across engines only for
    per-partition runs <= 16 KB (32 rows x 512 B here).
  - Partitions map to SDMA engines via an interleaved port map: [0:64)
    uses the 8 even engines, [64:128) the 8 odd ones. A single
    127/128-partition transfer runs its engines in near-lockstep with
    per-descriptor completion bookkeeping (~110 GB/s); TWO DMAs over
    disjoint halves sustain ~230 GB/s. All big transfers are issued as
    even/odd half-partition pairs.
"""

import numpy as np

import concourse.bacc as bacc
import concourse.mybir as mybir
import concourse.tile as tile
from concourse.bass_utils import run_bass_kernel_spmd

D = H = W = 128
HALF = 64
N_CORES = 8
HC = 32  # h-chunk size
F32 = mybir.dt.float32

# (od, oh, ow) per output channel
OFFSETS = [(0, -1, -1), (-1, 0, -1), (1, -1, -1), (-1, 1, -1), (-1, -1, 0), (-1, -1, 1)]

_NC_CACHE = {}


def build_nc(debug=False):
    nc = bacc.Bacc("TRN2", target_bir_lowering=False, debug=debug)
    x = nc.dram_tensor("x", [D, H, W], F32, kind="ExternalInput")
    out = nc.dram_tensor("out", [6, D, H, W], F32, kind="ExternalOutput")

    sub = mybir.AluOpType.subtract
    relu = mybir.ActivationFunctionType.Relu
    n_chunks = H // HC

    def split_dma(dst, src, dmax):
        # even-engine half then odd-engine half
        nc.gpsimd.dma_start(out=dst[0:HALF], in_=src[0:HALF])
        nc.gpsimd.dma_start(out=dst[HALF:dmax], in_=src[HALF:dmax])

    with tile.TileContext(nc) as tc:
        with (
            tc.tile_pool(name="xt", bufs=1) as xt_pool,
            tc.tile_pool(name="xp", bufs=1) as xp_pool,
            tc.tile_pool(name="och", bufs=6) as och_pool,
            tc.tile_pool(name="plane", bufs=2) as plane_pool,
        ):
            # x fully resident, loaded as 4 x 2 half-partition chunks
            xt = xt_pool.tile([D, H, W], F32)
            for c in range(n_chunks):
                hsl = slice(c * HC, (c + 1) * HC)
                split_dma(xt[:, hsl, :], x[:, hsl, :], D)

            # d-boundary planes: out[i, 0] = relu(x[0]) for od=-1 channels,
            # out[2, 127] = relu(x[127]); h on partitions so relu is wide.
            p0 = plane_pool.tile([H, W], F32)
            nc.sync.dma_start(out=p0[:], in_=x[0])
            nc.vector.tensor_scalar_max(p0[:], p0[:], 0.0)
            for i, (od, _, _) in enumerate(OFFSETS):
                if od == -1:
                    nc.sync.dma_start(out=out[i, 0], in_=p0[:])
            p1 = plane_pool.tile([H, W], F32)
            nc.sync.dma_start(out=p1[:], in_=x[D - 1])
            nc.vector.tensor_scalar_max(p1[:], p1[:], 0.0)
            nc.sync.dma_start(out=out[2, D - 1], in_=p1[:])

            for c in range(n_chunks):
                h0 = c * HC
                lo = max(0, h0 - 1)  # first xp row loaded (halo below)
                # xp rows [lo, h0+HC) on partitions 0..126; 32-row (16 KB)
                # halves + <=1-row tail on the HWDGE ring.
                xp = xp_pool.tile([D, HC + 1, W], F32)
                xpb = h0 - lo  # local row of absolute row h0
                nc.gpsimd.dma_start(
                    out=xp[0:HALF, 0:HC, :], in_=x[1 : HALF + 1, lo : lo + HC, :]
                )
                nc.gpsimd.dma_start(
                    out=xp[HALF : D - 1, 0:HC, :],
                    in_=x[HALF + 1 : D, lo : lo + HC, :],
                )
                if h0 + HC - lo > HC:
                    nc.sync.dma_start(
                        out=xp[0 : D - 1, HC : HC + 1, :],
                        in_=x[1:D, lo + HC : h0 + HC, :],
                    )

                def xprow(h):  # absolute h row -> local xp row
                    return h - h0 + xpb

                for i, (od, oh, ow) in enumerate(OFFSETS):
                    # A = operand aligned with the output partition frame,
                    # S = the d-shifted operand (reads at h+oh, w+ow).
                    dc = D if od == 0 else D - 1

                    hs = max(h0, -oh)
                    he = min(h0 + HC, H - max(0, oh))
                    ws = max(0, -ow)
                    we = W - max(0, ow)

                    if od == -1:  # substituted frame: A=xp, S=xt
                        in0 = xp[0:dc, xprow(hs) : xprow(he), ws:we]
                        in1 = xt[0:dc, hs + oh : he + oh, ws + ow : we + ow]
                    elif od == 1:  # A=xt, S=xp
                        in0 = xt[0:dc, hs:he, ws:we]
                        in1 = xp[
                            0:dc, xprow(hs + oh) : xprow(he + oh), ws + ow : we + ow
                        ]
                    else:
                        in0 = xt[0:dc, hs:he, ws:we]
                        in1 = xt[0:dc, hs + oh : he + oh, ws + ow : we + ow]

                    och = och_pool.tile([D, HC, W], F32)
                    nc.vector.tensor_tensor(
                        out=och[0:dc, hs - h0 : he - h0, ws:we],
                        in0=in0,
                        in1=in1,
                        op=sub,
                    )
                    # boundary strips (shifted source zero there -> relu(A));
                    # on ACT so the store depends on one engine's tail only.
                    def strip_src(hb_s, hb_e, wb_s, wb_e):
                        if od == -1:
                            return xp[0:dc, xprow(hb_s) : xprow(hb_e), wb_s:wb_e]
                        return xt[0:dc, hb_s:hb_e, wb_s:wb_e]

                    if oh == -1 and h0 == 0:
                        nc.scalar.activation(
                            och[0:dc, 0:1, :], strip_src(0, 1, 0, W), relu
                        )
                    if oh == 1 and h0 + HC == H:
                        nc.scalar.activation(
                            och[0:dc, HC - 1 : HC, :], strip_src(H - 1, H, 0, W), relu
                        )
                    if ow != 0:
                        wb = 0 if ow == -1 else W - 1
                        nc.scalar.activation(
                            och[0:dc, hs - h0 : he - h0, wb : wb + 1],
                            strip_src(hs, he, wb, wb + 1),
                            relu,
                        )
                    nc.scalar.activation(
                        och[0:dc, hs - h0 : he - h0, ws:we],
                        och[0:dc, hs - h0 : he - h0, ws:we],
                        relu,
                    )

                    if od == -1:
                        split_dma(out[i, 1:D, h0 : h0 + HC, :], och, D - 1)
                    elif od == 1:
                        split_dma(out[i, 0 : D - 1, h0 : h0 + HC, :], och, D - 1)
                    else:
                        split_dma(out[i, :, h0 : h0 + HC, :], och, D)

    nc.compile()
    return nc


def _get_nc():
    if "nc" not in _NC_CACHE:
        _NC_CACHE["nc"] = build_nc()
    return _NC_CACHE["nc"]


def kernel(x: np.ndarray) -> np.ndarray:
    assert x.shape == (N_CORES, 1, D, H, W), x.shape
    nc = _get_nc()
    in_maps = [{"x": np.ascontiguousarray(x[b, 0], dtype=np.float32)} for b in range(N_CORES)]
    res = run_bass_kernel_spmd(nc, in_maps, core_ids=list(range(N_CORES)))
    return np.stack([r["out"] for r in res.results], axis=0)


# revision 10
# speedup vs baseline: 4.7357x; 1.1562x over previous
"""Trainium2 Bass kernel for CubeFaceNN.

Computes, for x of shape [8, 1, 128, 128, 128] (f32):
    out[b, i, p] = relu(x[b, 0, p] - x[b, 0, p + OFF[i]])   (zero padded)
with OFF = [(0,-1,-1), (-1,0,-1), (1,-1,-1), (-1,1,-1), (-1,-1,0), (-1,-1,1)]
(derived from the reference's adj % 3 - 1 indexing).

Sharding: pure data parallel — batch b -> NeuronCore b (8 cores).

Per-core layout: depth d on the 128 SBUF partitions, (h, w) in the free
dims. x is fully resident in SBUF (64KB/partition); a partition-shifted
copy xp[d] = x[d+1] is loaded straight from HBM in prefetched h-chunks
(compute engines cannot address SBUF at a partition offset of 1).
Channels with od = -1 are computed in the substituted frame
    out[i, d'+1] = relu(xp[d'] - x[d', h+oh, w+ow])
so one shifted copy serves all five d-shifting channels; the d-boundary
faces are written from small [h, w]-layout plane tiles.

DMA rules learned from traces/probes on this silicon:
  - The HWDGE dynamic ring drains through a single SDMA engine
    (~27 GB/s) -> only tiny plane/tail transfers use nc.sync.
  - SWDGE (nc.gpsimd) spreads descriptors across engines only for
    per-partition runs <= 16 KB.
  - Partitions map to SDMA engines via an interleaved port map: [0:64)
    uses the 8 even engines, [64:128) the 8 odd ones. A single
    127/128-partition transfer runs its engines in near-lockstep with
    per-descriptor completion bookkeeping (~110 GB/s); TWO DMAs over
    disjoint halves sustain ~230 GB/s. All big transfers are issued as
    even/odd half-partition pairs.
"""

import numpy as np

import concourse.bacc as bacc
import concourse.mybir as mybir
import concourse.tile as tile
from concourse.bass_utils import run_bass_kernel_spmd

D = H = W = 128
HALF = 64
N_CORES = 8
HC = 16  # compute/store h-chunk
XC = 32  # xp load h-chunk
F32 = mybir.dt.float32

# (od, oh, ow) per output channel
OFFSETS = [(0, -1, -1), (-1, 0, -1), (1, -1, -1), (-1, 1, -1), (-1, -1, 0), (-1, -1, 1)]

_NC_CACHE = {}


def build_nc(debug=False):
    nc = bacc.Bacc("TRN2", target_bir_lowering=False, debug=debug)
    x = nc.dram_tensor("x", [D, H, W], F32, kind="ExternalInput")
    out = nc.dram_tensor("out", [6, D, H, W], F32, kind="ExternalOutput")

    sub = mybir.AluOpType.subtract
    relu = mybir.ActivationFunctionType.Relu
    n_chunks = H // HC

    def split_dma(dst, src, dmax):
        # even-engine half then odd-engine half
        nc.gpsimd.dma_start(out=dst[0:HALF], in_=src[0:HALF])
        nc.gpsimd.dma_start(out=dst[HALF:dmax], in_=src[HALF:dmax])

    with tile.TileContext(nc) as tc:
        with (
            tc.tile_pool(name="xt", bufs=1) as xt_pool,
            tc.tile_pool(name="xp", bufs=2) as xp_pool,
            tc.tile_pool(name="och", bufs=8) as och_pool,
            tc.tile_pool(name="plane", bufs=2) as plane_pool,
        ):
            # x fully resident, loaded as 4 x 2 half-partition chunks
            xt = xt_pool.tile([D, H, W], F32)
            for c in range(H // XC):
                hsl = slice(c * XC, (c + 1) * XC)
                split_dma(xt[:, hsl, :], x[:, hsl, :], D)

            def load_xp_chunk(cx):
                # xp rows [cx*XC - 1, cx*XC + XC) on partitions 0..126
                # (xp[d, r] = x[d+1, lo + r]); 32-row (16 KB) halves +
                # <=1-row tail on the HWDGE ring.
                lo = max(0, cx * XC - 1)
                hi = cx * XC + XC
                t = xp_pool.tile([D, XC + 1, W], F32)
                base = 1 if cx > 0 else 0  # local row of absolute row cx*XC
                nc.gpsimd.dma_start(
                    out=t[0:HALF, 0:XC, :], in_=x[1 : HALF + 1, lo : lo + XC, :]
                )
                nc.gpsimd.dma_start(
                    out=t[HALF : D - 1, 0:XC, :], in_=x[HALF + 1 : D, lo : lo + XC, :]
                )
                if hi - lo > XC:
                    nc.sync.dma_start(
                        out=t[0 : D - 1, XC : XC + 1, :], in_=x[1:D, lo + XC : hi, :]
                    )
                return t, base

            xp_tiles = {0: load_xp_chunk(0)}

            # d-boundary planes: out[i, 0] = relu(x[0]) for od=-1 channels,
            # out[2, 127] = relu(x[127]); h on partitions so relu is wide.
            p0 = plane_pool.tile([H, W], F32)
            nc.sync.dma_start(out=p0[:], in_=x[0])
            nc.vector.tensor_scalar_max(p0[:], p0[:], 0.0)
            for i, (od, _, _) in enumerate(OFFSETS):
                if od == -1:
                    nc.sync.dma_start(out=out[i, 0], in_=p0[:])
            p1 = plane_pool.tile([H, W], F32)
            nc.sync.dma_start(out=p1[:], in_=x[D - 1])
            nc.vector.tensor_scalar_max(p1[:], p1[:], 0.0)
            nc.sync.dma_start(out=out[2, D - 1], in_=p1[:])

            for c in range(n_chunks):
                h0 = c * HC
                cx = h0 // XC  # xp tile covering this compute chunk
                if h0 % XC == 0:
                    xp, xpb = xp_tiles.pop(cx)
                    if cx + 1 < H // XC:  # prefetch one XC block ahead
                        xp_tiles[cx + 1] = load_xp_chunk(cx + 1)
                x0 = cx * XC

                def xprow(h):  # absolute h row -> local xp row
                    return h - x0 + xpb

                for i, (od, oh, ow) in enumerate(OFFSETS):
                    # A = operand aligned with the output partition frame,
                    # S = the d-shifted operand (reads at h+oh, w+ow).
                    dc = D if od == 0 else D - 1

                    hs = max(h0, -oh)
                    he = min(h0 + HC, H - max(0, oh))
                    ws = max(0, -ow)
                    we = W - max(0, ow)

                    if od == -1:  # substituted frame: A=xp, S=xt
                        in0 = xp[0:dc, xprow(hs) : xprow(he), ws:we]
                        in1 = xt[0:dc, hs + oh : he + oh, ws + ow : we + ow]
                    elif od == 1:  # A=xt, S=xp
                        in0 = xt[0:dc, hs:he, ws:we]
                        in1 = xp[
                            0:dc, xprow(hs + oh) : xprow(he + oh), ws + ow : we + ow
                        ]
                    else:
                        in0 = xt[0:dc, hs:he, ws:we]
                        in1 = xt[0:dc, hs + oh : he + oh, ws + ow : we + ow]

                    och = och_pool.tile([D, HC, W], F32)
                    nc.vector.tensor_tensor(
                        out=och[0:dc, hs - h0 : he - h0, ws:we],
                        in0=in0,
                        in1=in1,
                        op=sub,
                    )
                    # boundary strips (shifted source zero there -> relu(A));
                    # on ACT so the store depends on one engine's tail only.
                    def strip_src(hb_s, hb_e, wb_s, wb_e):
                        if od == -1:
                            return xp[0:dc, xprow(hb_s) : xprow(hb_e), wb_s:wb_e]
                        return xt[0:dc, hb_s:hb_e, wb_s:wb_e]

                    if oh == -1 and h0 == 0:
                        nc.scalar.activation(
                            och[0:dc, 0:1, :], strip_src(0, 1, 0, W), relu
                        )
                    if oh == 1 and h0 + HC == H:
                        nc.scalar.activation(
                            och[0:dc, HC - 1 : HC, :], strip_src(H - 1, H, 0, W), relu
                        )
                    if ow != 0:
                        wb = 0 if ow == -1 else W - 1
                        nc.scalar.activation(
                            och[0:dc, hs - h0 : he - h0, wb : wb + 1],
                            strip_src(hs, he, wb, wb + 1),
                            relu,
                        )
                    nc.scalar.activation(
                        och[0:dc, hs - h0 : he - h0, ws:we],
                        och[0:dc, hs - h0 : he - h0, ws:we],
                        relu,
                    )

                    if od == -1:
                        split_dma(out[i, 1:D, h0 : h0 + HC, :], och, D - 1)
                    elif od == 1:
                        split_dma(out[i, 0 : D - 1, h0 : h0 + HC, :], och, D - 1)
                    else:
                        split_dma(out[i, :, h0 : h0 + HC, :], och, D)

    nc.compile()
    return nc


def _get_nc():
    if "nc" not in _NC_CACHE:
        _NC_CACHE["nc"] = build_nc()
    return _NC_CACHE["nc"]


def kernel(x: np.ndarray) -> np.ndarray:
    assert x.shape == (N_CORES, 1, D, H, W), x.shape
    nc = _get_nc()
    in_maps = [{"x": np.ascontiguousarray(x[b, 0], dtype=np.float32)} for b in range(N_CORES)]
    res = run_bass_kernel_spmd(nc, in_maps, core_ids=list(range(N_CORES)))
    return np.stack([r["out"] for r in res.results], axis=0)
